# revision 1
# baseline (speedup 1.0000x reference)
"""Binary conv + BN(train) + ReLU fused Trainium2 SPMD kernel.

Reference computation (NCHW, x:(32,256,56,56) f32):
    mean/var over (N,H,W) per channel; xn = (x-mean)*rsqrt(var+eps)*gamma+beta
    xb = sign(xn); wb = sign(W); y = relu(conv3x3(xb, wb, pad=1) + bias)

Strategy: data-parallel over batch across 8 NeuronCores (4 images each).
Per-core partial BN stats (bn_stats/bn_aggr on DVE) are combined with a
2KB AllReduce; normalize+sign runs as one scalar-engine activation
(Sign(a*x+b)) writing bf16 into zero-padded 58x58 planes; the 3x3 conv is
18 accumulating 128x128xN matmuls per output tile (ci-chunks x taps) on the
tensor engine; bias+relu fused into the PSUM->SBUF drain on ScalarE.
Sign values are exactly representable in bf16 and PSUM accumulates in fp32,
so the binarized conv is exact.
"""

import sys

for _p in ("/opt/trn_rl_repo", "/root/.axon_site/_ro/trn_rl_repo"):
    if _p not in sys.path:
        sys.path.append(_p)

import numpy as np
import ml_dtypes

import concourse.bass as bass
import concourse.mybir as mybir
import concourse.tile as tile
from concourse import bacc, bass_utils

F32 = mybir.dt.float32
BF16 = mybir.dt.bfloat16
AF = mybir.ActivationFunctionType

N_CORES = 8
NB = 4          # images per core
C = 256
P = 128         # partitions / chunk size
NCH = 2         # channel chunks (ci and co)
H = W = 56
HW = H * W      # 3136
PH = PW = 58    # padded plane
PSZ = PH * PW   # 3364
RG = 8          # output rows per psum tile
NG = H // RG    # 7 row groups
NT = RG * W     # 448 columns per matmul
BN_EPS = 1e-5
BLK = 4         # psum tiles in flight per weight reuse block

_CACHE = {}


def _build_nc():
    nc = bacc.Bacc("TRN2", target_bir_lowering=False, debug=False,
                   num_devices=N_CORES)
    xs = nc.dram_tensor("xs", [NB, C, H, W], F32, kind="ExternalInput")
    wt = nc.dram_tensor("wt", [NCH, P, 9, NCH, P], BF16, kind="ExternalInput")
    par = nc.dram_tensor("par", [NCH, P, 3], F32, kind="ExternalInput")
    ys = nc.dram_tensor("ys", [NB, C, H, W], F32, kind="ExternalOutput")

    with tile.TileContext(nc) as tc:
        with (
            tc.tile_pool(name="main", bufs=1) as main,
            tc.tile_pool(name="outp", bufs=4) as outp,
            tc.tile_pool(name="psum", bufs=8, space="PSUM") as psum,
            tc.tile_pool(name="dram", bufs=1, space="DRAM") as dram,
        ):
            xt = [main.tile([P, NB * HW], F32, name=f"xt{c}") for c in range(NCH)]
            xb = [main.tile([P, NB * PSZ], BF16, name=f"xb{c}") for c in range(NCH)]
            wb = [main.tile([P, 9 * NCH * P], BF16, name=f"wb{c}") for c in range(NCH)]
            parb = [main.tile([P, 3], F32, name=f"parb{c}") for c in range(NCH)]
            st6 = [main.tile([P, NB * 7 * 6], F32, name=f"st6{c}") for c in range(NCH)]

            for c in range(NCH):
                nc.sync.dma_start(wb[c][:], wt[c].rearrange("p t o m -> p (t o m)"))
                nc.sync.dma_start(parb[c][:], par[c])
                # zero padded sign planes (borders stay 0 = conv padding)
                nc.vector.memset(xb[c][:], 0.0)

            # load x (channels on partitions) + one-pass partial stats
            for c in range(NCH):
                for n in range(NB):
                    nc.sync.dma_start(
                        xt[c][:, n * HW:(n + 1) * HW],
                        xs[n, c * P:(c + 1) * P].rearrange("p h w -> p (h w)"),
                    )
                    for g in range(7):
                        nc.vector.bn_stats(
                            st6[c][:, (n * 7 + g) * 6:(n * 7 + g + 1) * 6],
                            xt[c][:, n * HW + g * NT: n * HW + (g + 1) * NT],
                        )

            # per-core (mean, var) -> (mean/8, E[x^2]/8) for the all-reduce
            mv = main.tile([P, 2 * NCH], F32)
            pre = main.tile([P, 2 * NCH], F32)
            t_a = main.tile([P, 1], F32)
            t_b = main.tile([P, 1], F32)
            for c in range(NCH):
                nc.vector.bn_aggr(mv[:, 2 * c:2 * c + 2], st6[c][:])
                mean = mv[:, 2 * c:2 * c + 1]
                var = mv[:, 2 * c + 1:2 * c + 2]
                nc.vector.tensor_mul(t_a[:], mean, mean)
                nc.vector.tensor_add(t_b[:], var, t_a[:])
                nc.vector.tensor_scalar_mul(pre[:, 2 * c:2 * c + 1], mean, 1.0 / N_CORES)
                nc.vector.tensor_scalar_mul(pre[:, 2 * c + 1:2 * c + 2], t_b[:], 1.0 / N_CORES)

            cc_in = dram.tile([P, 2 * NCH], F32)
            cc_out = dram.tile([P, 2 * NCH], F32)
            nc.sync.dma_start(cc_in[:], pre[:])
            nc.gpsimd.collective_compute(
                "AllReduce",
                mybir.AluOpType.add,
                replica_groups=[list(range(N_CORES))],
                ins=[cc_in[:].opt()],
                outs=[cc_out[:].opt()],
            )
            gs = main.tile([P, 2 * NCH], F32)
            nc.sync.dma_start(gs[:], cc_out[:])

            # a = gamma*rsqrt(var+eps), b = beta - mean*a  (per channel)
            ab = main.tile([P, 2 * NCH], F32)
            u1 = main.tile([P, 1], F32)
            u2 = main.tile([P, 1], F32)
            u3 = main.tile([P, 1], F32)
            for c in range(NCH):
                gmean = gs[:, 2 * c:2 * c + 1]
                ex2 = gs[:, 2 * c + 1:2 * c + 2]
                a_ap = ab[:, 2 * c:2 * c + 1]
                b_ap = ab[:, 2 * c + 1:2 * c + 2]
                nc.vector.tensor_mul(u1[:], gmean, gmean)
                nc.vector.tensor_sub(u2[:], ex2, u1[:])          # global var
                nc.vector.tensor_scalar_add(u2[:], u2[:], BN_EPS)
                nc.scalar.activation(u3[:], u2[:], AF.Sqrt)
                nc.vector.reciprocal(u1[:], u3[:])               # rsqrt
                nc.vector.tensor_mul(a_ap, parb[c][:, 0:1], u1[:])
                nc.vector.tensor_mul(u2[:], gmean, a_ap)
                nc.vector.tensor_sub(b_ap, parb[c][:, 1:2], u2[:])

            # normalize + sign -> padded bf16 planes
            xbv = [xb[c].rearrange("p (n h w) -> p n h w", n=NB, h=PH) for c in range(NCH)]
            for n in range(NB):
                for c in range(NCH):
                    nc.scalar.activation(
                        xbv[c][:, n, 1:1 + H, 1:1 + W],
                        xt[c][:, n * HW:(n + 1) * HW].rearrange("p (h w) -> p h w", w=W),
                        AF.Sign,
                        bias=ab[:, 2 * c + 1:2 * c + 2],
                        scale=ab[:, 2 * c:2 * c + 1],
                    )

            # 3x3 binary conv: 18 accumulating matmuls per [co_chunk, 8x56] tile
            jobs = [(n, g) for n in range(NB) for g in range(NG)]
            for o in range(NCH):
                for blk_start in range(0, len(jobs), BLK):
                    blk = jobs[blk_start:blk_start + BLK]
                    pts = [psum.tile([P, NT], F32, name="ps", tag="ps") for _ in blk]
                    for c in range(NCH):
                        for t in range(9):
                            ky, kx = divmod(t, 3)
                            w_ap = wb[c][:, (t * NCH + o) * P:(t * NCH + o + 1) * P]
                            first = (c == 0 and t == 0)
                            last = (c == NCH - 1 and t == 8)
                            for k, (n, g) in enumerate(blk):
                                rhs = xbv[c][:, n, g * RG + ky: g * RG + ky + RG, kx:kx + W]
                                nc.tensor.matmul(pts[k][:], w_ap, rhs,
                                                 start=first, stop=last)
                    for k, (n, g) in enumerate(blk):
                        ob = outp.tile([P, NT], F32, name="ob", tag="ob")
                        nc.scalar.activation(ob[:], pts[k][:], AF.Relu,
                                             bias=parb[o][:, 2:3])
                        nc.sync.dma_start(
                            ys[n, o * P:(o + 1) * P, g * RG:(g + 1) * RG, :],
                            ob.rearrange("p (h w) -> p h w", w=W),
                        )
    nc.compile()
    return nc


def _get_nc():
    if "nc" not in _CACHE:
        _CACHE["nc"] = _build_nc()
    return _CACHE["nc"]


def _prep_inputs(x, gamma, beta, weight, bias):
    # weights: sign -> bf16, laid out [ci_chunk, ci_in, tap, co_chunk, co_in]
    wsign = np.sign(weight.astype(np.float32))
    wT = (
        wsign.reshape(NCH, P, NCH, P, 3, 3)      # o, m, c, p, ky, kx
        .transpose(2, 3, 4, 5, 0, 1)             # c, p, ky, kx, o, m
        .reshape(NCH, P, 9, NCH, P)
        .astype(ml_dtypes.bfloat16)
    )
    par = np.stack(
        [gamma.astype(np.float32), beta.astype(np.float32), bias.astype(np.float32)],
        axis=-1,
    ).reshape(NCH, P, 3)
    x = np.ascontiguousarray(x, dtype=np.float32)
    in_maps = [
        {"xs": x[j * NB:(j + 1) * NB], "wt": wT, "par": par}
        for j in range(N_CORES)
    ]
    return in_maps


def _run(x, gamma, beta, weight, bias, trace=False):
    nc = _get_nc()
    in_maps = _prep_inputs(x, gamma, beta, weight, bias)
    res = bass_utils.run_bass_kernel_spmd(
        nc, in_maps, core_ids=list(range(N_CORES)), trace=trace
    )
    out = np.concatenate([res.results[j]["ys"] for j in range(N_CORES)], axis=0)
    return out, res


def kernel(x, gamma, beta, weight, bias):
    out, _ = _run(x, gamma, beta, weight, bias, trace=False)
    return out


# revision 2
# speedup vs baseline: 1.4593x; 1.4593x over previous
"""Binary conv + BN(train) + ReLU fused Trainium2 SPMD kernel.

Reference computation (NCHW, x:(32,256,56,56) f32):
    mean/var over (N,H,W) per channel; xn = (x-mean)*rsqrt(var+eps)*gamma+beta
    xb = sign(xn); wb = sign(W); y = relu(conv3x3(xb, wb, pad=1) + bias)

Strategy: data-parallel over batch across 8 NeuronCores (4 images each).
Per-core partial BN stats (bn_stats/bn_aggr on DVE, pipelined with the x
load) are combined with a 2KB AllReduce; normalize+sign runs as one
scalar-engine activation (Sign(a*x+b)) writing fp8 into zero-padded 58x58
planes; the 3x3 conv is 9 accumulating DoubleRow fp8 matmuls (K=256 via
the paired-row mode) per 128x448 output tile; bias+relu is fused into the
PSUM->SBUF drain on ScalarE. Sign values are exact in fp8/bf16 and PSUM
accumulates in fp32, so the binarized conv is exact.
"""

import sys

for _p in ("/opt/trn_rl_repo", "/root/.axon_site/_ro/trn_rl_repo"):
    if _p not in sys.path:
        sys.path.append(_p)

import numpy as np

import concourse.bass as bass
import concourse.mybir as mybir
import concourse.tile as tile
from concourse import bacc, bass_utils

F32 = mybir.dt.float32
BF16 = mybir.dt.bfloat16
FP8 = mybir.dt.float8e4
AF = mybir.ActivationFunctionType

N_CORES = 8
NB = 4          # images per core
C = 256
P = 128         # partitions / chunk size
NCH = 2         # channel chunks (ci and co)
H = W = 56
HW = H * W      # 3136
PH = PW = 58    # padded plane
PSZ = PH * PW   # 3364
RG = 8          # output rows per psum tile
NG = H // RG    # 7 row groups
NT = RG * W     # 448 columns per matmul
BN_EPS = 1e-5
BLK = 4         # psum tiles in flight per weight-reuse block

USE_FP8 = True

_CACHE = {}


def _build_nc():
    act_dt = FP8 if USE_FP8 else BF16
    nc = bacc.Bacc("TRN2", target_bir_lowering=False, debug=False,
                   num_devices=N_CORES)
    xs = nc.dram_tensor("xs", [NB, C, H, W], F32, kind="ExternalInput")
    if USE_FP8:
        wt = nc.dram_tensor("wt", [P, NCH, 9 * NCH * P], FP8, kind="ExternalInput")
    else:
        wt = nc.dram_tensor("wt", [NCH, P, 9, NCH, P], BF16, kind="ExternalInput")
    par = nc.dram_tensor("par", [NCH, P, 3], F32, kind="ExternalInput")
    ys = nc.dram_tensor("ys", [NB, C, H, W], F32, kind="ExternalOutput")

    with tile.TileContext(nc) as tc:
        with (
            tc.tile_pool(name="main", bufs=1) as main,
            tc.tile_pool(name="outp", bufs=4) as outp,
            tc.tile_pool(name="psum", bufs=8, space="PSUM") as psum,
            tc.tile_pool(name="dram", bufs=1, space="DRAM") as dram,
        ):
            xt = [main.tile([P, NB * HW], F32, name=f"xt{c}") for c in range(NCH)]
            # sign planes: [p, ci_chunk, image, padded 58x58] (chunk dim = fp8
            # DoubleRow pair dim)
            xball = main.tile([P, NCH, NB * PSZ], act_dt, name="xball")
            xbv = xball.rearrange("p j (n h w) -> p j n h w", n=NB, h=PH)
            if USE_FP8:
                wb = main.tile([P, NCH, 9 * NCH * P], FP8, name="wb")
            else:
                wb = main.tile([P, NCH, 9 * NCH * P], BF16, name="wb")
            parb = [main.tile([P, 3], F32, name=f"parb{c}") for c in range(NCH)]
            st6 = [main.tile([P, NB * 7 * 6], F32, name=f"st6{c}") for c in range(NCH)]

            # load x (channels on partitions) + one-pass partial stats,
            # pipelined per (image, chunk)
            for n in range(NB):
                for c in range(NCH):
                    nc.sync.dma_start(
                        xt[c][:, n * HW:(n + 1) * HW],
                        xs[n, c * P:(c + 1) * P].rearrange("p h w -> p (h w)"),
                    )
            if USE_FP8:
                nc.sync.dma_start(wb[:], wt[:])
            else:
                nc.sync.dma_start(
                    wb[:],
                    wt.rearrange("c p t o m -> p c (t o m)"),
                )
            for c in range(NCH):
                nc.sync.dma_start(parb[c][:], par[c])

            # zero only the pad borders of the sign planes (GpSimd; interior
            # is fully overwritten by the Sign activation)
            for c in range(NCH):
                for n in range(NB):
                    nc.gpsimd.memset(xbv[:, c, n, 0, :], 0.0)
                    nc.gpsimd.memset(xbv[:, c, n, PH - 1, :], 0.0)
                    nc.gpsimd.memset(xbv[:, c, n, 1:PH - 1, 0], 0.0)
                    nc.gpsimd.memset(xbv[:, c, n, 1:PH - 1, PW - 1], 0.0)

            for n in range(NB):
                for c in range(NCH):
                    for g in range(7):
                        nc.vector.bn_stats(
                            st6[c][:, (n * 7 + g) * 6:(n * 7 + g + 1) * 6],
                            xt[c][:, n * HW + g * NT: n * HW + (g + 1) * NT],
                        )

            # per-core (mean, var) -> (mean/8, E[x^2]/8) for the all-reduce
            mv = main.tile([P, 2 * NCH], F32)
            pre = main.tile([P, 2 * NCH], F32)
            t_a = main.tile([P, 1], F32)
            t_b = main.tile([P, 1], F32)
            for c in range(NCH):
                nc.vector.bn_aggr(mv[:, 2 * c:2 * c + 2], st6[c][:])
                mean = mv[:, 2 * c:2 * c + 1]
                var = mv[:, 2 * c + 1:2 * c + 2]
                nc.vector.tensor_mul(t_a[:], mean, mean)
                nc.vector.tensor_add(t_b[:], var, t_a[:])
                nc.vector.tensor_scalar_mul(pre[:, 2 * c:2 * c + 1], mean, 1.0 / N_CORES)
                nc.vector.tensor_scalar_mul(pre[:, 2 * c + 1:2 * c + 2], t_b[:], 1.0 / N_CORES)

            cc_in = dram.tile([P, 2 * NCH], F32)
            cc_out = dram.tile([P, 2 * NCH], F32)
            nc.sync.dma_start(cc_in[:], pre[:])
            nc.gpsimd.collective_compute(
                "AllReduce",
                mybir.AluOpType.add,
                replica_groups=[list(range(N_CORES))],
                ins=[cc_in[:].opt()],
                outs=[cc_out[:].opt()],
            )
            gs = main.tile([P, 2 * NCH], F32)
            nc.sync.dma_start(gs[:], cc_out[:])

            # a = gamma*rsqrt(var+eps), b = beta - mean*a  (per channel)
            ab = main.tile([P, 2 * NCH], F32)
            u1 = main.tile([P, 1], F32)
            u2 = main.tile([P, 1], F32)
            u3 = main.tile([P, 1], F32)
            for c in range(NCH):
                gmean = gs[:, 2 * c:2 * c + 1]
                ex2 = gs[:, 2 * c + 1:2 * c + 2]
                a_ap = ab[:, 2 * c:2 * c + 1]
                b_ap = ab[:, 2 * c + 1:2 * c + 2]
                nc.vector.tensor_mul(u1[:], gmean, gmean)
                nc.vector.tensor_sub(u2[:], ex2, u1[:])          # global var
                nc.vector.tensor_scalar_add(u2[:], u2[:], BN_EPS)
                nc.scalar.activation(u3[:], u2[:], AF.Sqrt)
                nc.vector.reciprocal(u1[:], u3[:])               # rsqrt
                nc.vector.tensor_mul(a_ap, parb[c][:, 0:1], u1[:])
                nc.vector.tensor_mul(u2[:], gmean, a_ap)
                nc.vector.tensor_sub(b_ap, parb[c][:, 1:2], u2[:])

            # normalize + sign -> padded planes
            for n in range(NB):
                for c in range(NCH):
                    nc.scalar.activation(
                        xbv[:, c, n, 1:1 + H, 1:1 + W],
                        xt[c][:, n * HW:(n + 1) * HW].rearrange("p (h w) -> p h w", w=W),
                        AF.Sign,
                        bias=ab[:, 2 * c + 1:2 * c + 2],
                        scale=ab[:, 2 * c:2 * c + 1],
                    )

            # 3x3 binary conv
            jobs = [(n, g) for n in range(NB) for g in range(NG)]
            for o in range(NCH):
                for blk_start in range(0, len(jobs), BLK):
                    blk = jobs[blk_start:blk_start + BLK]
                    pts = [psum.tile([P, NT], F32, name="ps", tag="ps") for _ in blk]
                    if USE_FP8:
                        for t in range(9):
                            ky, kx = divmod(t, 3)
                            w_ap = wb[:, :, (t * NCH + o) * P:(t * NCH + o + 1) * P]
                            for k, (n, g) in enumerate(blk):
                                rhs = xbv[:, :, n, g * RG + ky: g * RG + ky + RG, kx:kx + W]
                                nc.tensor.matmul(
                                    pts[k][:], w_ap, rhs,
                                    start=(t == 0), stop=(t == 8),
                                    perf_mode=mybir.MatmulPerfMode.DoubleRow,
                                )
                    else:
                        for c in range(NCH):
                            for t in range(9):
                                ky, kx = divmod(t, 3)
                                w_ap = wb[:, c, (t * NCH + o) * P:(t * NCH + o + 1) * P]
                                first = (c == 0 and t == 0)
                                last = (c == NCH - 1 and t == 8)
                                for k, (n, g) in enumerate(blk):
                                    rhs = xbv[:, c, n, g * RG + ky: g * RG + ky + RG, kx:kx + W]
                                    nc.tensor.matmul(pts[k][:], w_ap, rhs,
                                                     start=first, stop=last)
                    for k, (n, g) in enumerate(blk):
                        ob = outp.tile([P, NT], F32, name="ob", tag="ob")
                        nc.scalar.activation(ob[:], pts[k][:], AF.Relu,
                                             bias=parb[o][:, 2:3])
                        nc.sync.dma_start(
                            ys[n, o * P:(o + 1) * P, g * RG:(g + 1) * RG, :],
                            ob.rearrange("p (h w) -> p h w", w=W),
                        )
    nc.compile()
    return nc


def _get_nc():
    if "nc" not in _CACHE:
        _CACHE["nc"] = _build_nc()
    return _CACHE["nc"]


def _prep_inputs(x, gamma, beta, weight, bias):
    wsign = np.sign(weight.astype(np.float32))
    if USE_FP8:
        # [p(ci_in), j(ci_chunk), (tap, o_chunk, co_in)]
        wT = (
            wsign.reshape(NCH, P, NCH, P, 3, 3)      # o, m, c, p, ky, kx
            .transpose(3, 2, 4, 5, 0, 1)             # p, c, ky, kx, o, m
            .reshape(P, NCH, 9 * NCH * P)
            .astype(mybir.dt.np(FP8))
        )
    else:
        wT = (
            wsign.reshape(NCH, P, NCH, P, 3, 3)      # o, m, c, p, ky, kx
            .transpose(2, 3, 4, 5, 0, 1)             # c, p, ky, kx, o, m
            .reshape(NCH, P, 9, NCH, P)
            .astype(mybir.dt.np(BF16))
        )
    par = np.stack(
        [gamma.astype(np.float32), beta.astype(np.float32), bias.astype(np.float32)],
        axis=-1,
    ).reshape(NCH, P, 3)
    x = np.ascontiguousarray(x, dtype=np.float32)
    in_maps = [
        {"xs": x[j * NB:(j + 1) * NB], "wt": wT, "par": par}
        for j in range(N_CORES)
    ]
    return in_maps


def _run(x, gamma, beta, weight, bias, trace=False):
    nc = _get_nc()
    in_maps = _prep_inputs(x, gamma, beta, weight, bias)
    res = bass_utils.run_bass_kernel_spmd(
        nc, in_maps, core_ids=list(range(N_CORES)), trace=trace
    )
    out = np.concatenate([res.results[j]["ys"] for j in range(N_CORES)], axis=0)
    return out, res


def kernel(x, gamma, beta, weight, bias):
    out, _ = _run(x, gamma, beta, weight, bias, trace=False)
    return out


# revision 10
# speedup vs baseline: 1.5656x; 1.0728x over previous
"""Binary conv + BN(train) + ReLU fused Trainium2 SPMD kernel.

Reference computation (NCHW, x:(32,256,56,56) f32):
    mean/var over (N,H,W) per channel; xn = (x-mean)*rsqrt(var+eps)*gamma+beta
    xb = sign(xn); wb = sign(W); y = relu(conv3x3(xb, wb, pad=1) + bias)

Strategy: data-parallel over batch across 8 NeuronCores (4 images each).
Per-core partial BN stats (bn_stats/bn_aggr on DVE, pipelined with the x
load) are combined with a 2KB AllReduce; normalize+sign runs as one
scalar-engine activation (Sign(a*x+b)) writing fp8 into zero-padded 58x58
planes; the 3x3 conv is 9 accumulating DoubleRow fp8 matmuls (K=256 via
the paired-row mode) per 128x448 output tile; bias+relu is fused into the
PSUM->SBUF drain on ScalarE. Sign values are exact in fp8/bf16 and PSUM
accumulates in fp32, so the binarized conv is exact.
"""

import sys

for _p in ("/opt/trn_rl_repo", "/root/.axon_site/_ro/trn_rl_repo"):
    if _p not in sys.path:
        sys.path.append(_p)

import numpy as np

import concourse.bass as bass
import concourse.mybir as mybir
import concourse.tile as tile
from concourse import bacc, bass_utils

F32 = mybir.dt.float32
BF16 = mybir.dt.bfloat16
FP8 = mybir.dt.float8e4
AF = mybir.ActivationFunctionType

N_CORES = 8
NB = 4          # images per core
C = 256
P = 128         # partitions / chunk size
NCH = 2         # channel chunks (ci and co)
H = W = 56
HW = H * W      # 3136
PH = PW = 58    # padded plane
PSZ = PH * PW   # 3364
RG = 8          # output rows per psum tile
NG = H // RG    # 7 row groups
NT = RG * W     # 448 columns per matmul
BN_EPS = 1e-5
BLK = 8         # psum tiles in flight per weight-reuse block

USE_FP8 = True
ELIDE_LDW = True  # skip redundant weight reloads within a weight-reuse block

_CACHE = {}


def _build_nc():
    act_dt = FP8 if USE_FP8 else BF16
    nc = bacc.Bacc("TRN2", target_bir_lowering=False, debug=False,
                   num_devices=N_CORES)
    xs = nc.dram_tensor("xs", [NB, C, H, W], F32, kind="ExternalInput")
    if USE_FP8:
        wt = nc.dram_tensor("wt", [P, NCH, 9 * NCH * P], FP8, kind="ExternalInput")
    else:
        wt = nc.dram_tensor("wt", [NCH, P, 9, NCH, P], BF16, kind="ExternalInput")
    par = nc.dram_tensor("par", [NCH, P, 3], F32, kind="ExternalInput")
    ys = nc.dram_tensor("ys", [NB, C, H, W], F32, kind="ExternalOutput")

    with tile.TileContext(nc) as tc:
        with (
            tc.tile_pool(name="main", bufs=1) as main,
            tc.tile_pool(name="outp", bufs=4) as outp,
            tc.tile_pool(name="psum", bufs=8, space="PSUM") as psum,
            tc.tile_pool(name="dram", bufs=1, space="DRAM") as dram,
        ):
            xt = [main.tile([P, NB * HW], F32, name=f"xt{c}") for c in range(NCH)]
            # sign planes: [p, ci_chunk, image, padded 58x58] (chunk dim = fp8
            # DoubleRow pair dim)
            xball = main.tile([P, NCH, NB * PSZ], act_dt, name="xball")
            xbv = xball.rearrange("p j (n h w) -> p j n h w", n=NB, h=PH)
            if USE_FP8:
                wb = main.tile([P, NCH, 9 * NCH * P], FP8, name="wb")
            else:
                wb = main.tile([P, NCH, 9 * NCH * P], BF16, name="wb")
            parc = main.tile([P, 3 * NCH], F32, name="parc")  # [gamma,beta,bias] x chunk
            st6 = [main.tile([P, NB * 7 * 6], F32, name=f"st6{c}") for c in range(NCH)]

            # load x (channels on partitions) + one-pass partial stats,
            # pipelined per (image, chunk)
            for n in range(NB):
                for c in range(NCH):
                    nc.sync.dma_start(
                        xt[c][:, n * HW:(n + 1) * HW],
                        xs[n, c * P:(c + 1) * P].rearrange("p h w -> p (h w)"),
                    )
            if USE_FP8:
                nc.sync.dma_start(wb[:], wt[:])
            else:
                nc.sync.dma_start(
                    wb[:],
                    wt.rearrange("c p t o m -> p c (t o m)"),
                )
            nc.sync.dma_start(
                parc.rearrange("p (c s) -> p c s", s=3),
                par.rearrange("c p s -> p c s"),
            )

            # zero only the pad borders of the sign planes (GpSimd; interior
            # is fully overwritten by the Sign activation)
            for c in range(NCH):
                for n in range(NB):
                    nc.gpsimd.memset(xbv[:, c, n, 0, :], 0.0)
                    nc.gpsimd.memset(xbv[:, c, n, PH - 1, :], 0.0)
                    nc.gpsimd.memset(xbv[:, c, n, 1:PH - 1, 0], 0.0)
                    nc.gpsimd.memset(xbv[:, c, n, 1:PH - 1, PW - 1], 0.0)

            for n in range(NB):
                for c in range(NCH):
                    for g in range(7):
                        nc.vector.bn_stats(
                            st6[c][:, (n * 7 + g) * 6:(n * 7 + g + 1) * 6],
                            xt[c][:, n * HW + g * NT: n * HW + (g + 1) * NT],
                        )

            # per-core (mean, var) -> (mean/8, E[x^2]/8) for the all-reduce
            mv = main.tile([P, 2 * NCH], F32)
            pre = main.tile([P, 2 * NCH], F32)
            t_a = main.tile([P, 1], F32)
            t_b = main.tile([P, 1], F32)
            for c in range(NCH):
                nc.vector.bn_aggr(mv[:, 2 * c:2 * c + 2], st6[c][:])
                mean = mv[:, 2 * c:2 * c + 1]
                var = mv[:, 2 * c + 1:2 * c + 2]
                nc.vector.tensor_mul(t_a[:], mean, mean)
                nc.vector.tensor_add(t_b[:], var, t_a[:])
                nc.vector.tensor_scalar_mul(pre[:, 2 * c:2 * c + 1], mean, 1.0 / N_CORES)
                nc.vector.tensor_scalar_mul(pre[:, 2 * c + 1:2 * c + 2], t_b[:], 1.0 / N_CORES)

            cc_in = dram.tile([P, 2 * NCH], F32)
            cc_out = dram.tile([P, 2 * NCH], F32)
            nc.sync.dma_start(cc_in[:], pre[:])
            nc.gpsimd.collective_compute(
                "AllReduce",
                mybir.AluOpType.add,
                replica_groups=[list(range(N_CORES))],
                ins=[cc_in[:].opt()],
                outs=[cc_out[:].opt()],
            )
            gs = main.tile([P, 2 * NCH], F32)
            nc.sync.dma_start(gs[:], cc_out[:])

            # a = gamma*rsqrt(var+eps), b = beta - mean*a, both chunks at once
            # layouts: gs = [m0,e0,m1,e1]; ab = [a0,a1,b0,b1]
            ab = main.tile([P, 2 * NCH], F32)
            u1 = main.tile([P, NCH], F32)
            u2 = main.tile([P, NCH], F32)
            gsv = gs.rearrange("p (c s) -> p c s", s=2)
            gmean = gsv[:, :, 0]
            ex2 = gsv[:, :, 1]
            parv = parc.rearrange("p (c s) -> p c s", s=3)
            av = ab[:, 0:NCH]
            bv = ab[:, NCH:2 * NCH]
            nc.vector.tensor_mul(u1[:], gmean, gmean)
            nc.vector.tensor_sub(u2[:], ex2, u1[:])          # global var
            nc.vector.tensor_scalar_add(u2[:], u2[:], BN_EPS)
            nc.scalar.activation(u1[:], u2[:], AF.Sqrt)
            nc.vector.reciprocal(u2[:], u1[:])               # rsqrt
            nc.vector.tensor_mul(av, parv[:, :, 0], u2[:])
            nc.vector.tensor_mul(u1[:], gmean, av)
            nc.vector.tensor_sub(bv, parv[:, :, 1], u1[:])

            # normalize + sign -> padded planes
            for n in range(NB):
                for c in range(NCH):
                    nc.scalar.activation(
                        xbv[:, c, n, 1:1 + H, 1:1 + W],
                        xt[c][:, n * HW:(n + 1) * HW].rearrange("p (h w) -> p h w", w=W),
                        AF.Sign,
                        bias=ab[:, NCH + c:NCH + c + 1],
                        scale=ab[:, c:c + 1],
                    )

            # 3x3 binary conv
            jobs = [(n, g) for n in range(NB) for g in range(NG)]
            for o in range(NCH):
                for blk_start in range(0, len(jobs), BLK):
                    blk = jobs[blk_start:blk_start + BLK]
                    pts = [psum.tile([P, NT], F32, name="ps", tag="ps") for _ in blk]
                    if USE_FP8:
                        for t in range(9):
                            ky, kx = divmod(t, 3)
                            w_ap = wb[:, :, (t * NCH + o) * P:(t * NCH + o + 1) * P]
                            for k, (n, g) in enumerate(blk):
                                rhs = xbv[:, :, n, g * RG + ky: g * RG + ky + RG, kx:kx + W]
                                mm = nc.tensor.matmul(
                                    pts[k][:], w_ap, rhs,
                                    start=(t == 0), stop=(t == 8),
                                    perf_mode=mybir.MatmulPerfMode.DoubleRow,
                                )
                                if ELIDE_LDW and k > 0:
                                    mm.ins.ldweights = False
                    else:
                        for c in range(NCH):
                            for t in range(9):
                                ky, kx = divmod(t, 3)
                                w_ap = wb[:, c, (t * NCH + o) * P:(t * NCH + o + 1) * P]
                                first = (c == 0 and t == 0)
                                last = (c == NCH - 1 and t == 8)
                                for k, (n, g) in enumerate(blk):
                                    rhs = xbv[:, c, n, g * RG + ky: g * RG + ky + RG, kx:kx + W]
                                    mm = nc.tensor.matmul(pts[k][:], w_ap, rhs,
                                                          start=first, stop=last)
                                    if ELIDE_LDW and k > 0:
                                        mm.ins.ldweights = False
                    for k, (n, g) in enumerate(blk):
                        ob = outp.tile([P, NT], F32, name="ob", tag="ob")
                        nc.scalar.activation(ob[:], pts[k][:], AF.Relu,
                                             bias=parc[:, 3 * o + 2:3 * o + 3])
                        nc.sync.dma_start(
                            ys[n, o * P:(o + 1) * P, g * RG:(g + 1) * RG, :],
                            ob.rearrange("p (h w) -> p h w", w=W),
                        )
    nc.compile()
    return nc


def _get_nc():
    if "nc" not in _CACHE:
        _CACHE["nc"] = _build_nc()
    return _CACHE["nc"]


def _prep_inputs(x, gamma, beta, weight, bias):
    wsign = np.sign(weight.astype(np.float32))
    if USE_FP8:
        # [p(ci_in), j(ci_chunk), (tap, o_chunk, co_in)]
        wT = (
            wsign.reshape(NCH, P, NCH, P, 3, 3)      # o, m, c, p, ky, kx
            .transpose(3, 2, 4, 5, 0, 1)             # p, c, ky, kx, o, m
            .reshape(P, NCH, 9 * NCH * P)
            .astype(mybir.dt.np(FP8))
        )
    else:
        wT = (
            wsign.reshape(NCH, P, NCH, P, 3, 3)      # o, m, c, p, ky, kx
            .transpose(2, 3, 4, 5, 0, 1)             # c, p, ky, kx, o, m
            .reshape(NCH, P, 9, NCH, P)
            .astype(mybir.dt.np(BF16))
        )
    par = np.stack(
        [gamma.astype(np.float32), beta.astype(np.float32), bias.astype(np.float32)],
        axis=-1,
    ).reshape(NCH, P, 3)
    x = np.ascontiguousarray(x, dtype=np.float32)
    in_maps = [
        {"xs": x[j * NB:(j + 1) * NB], "wt": wT, "par": par}
        for j in range(N_CORES)
    ]
    return in_maps


def _run(x, gamma, beta, weight, bias, trace=False):
    nc = _get_nc()
    in_maps = _prep_inputs(x, gamma, beta, weight, bias)
    res = bass_utils.run_bass_kernel_spmd(
        nc, in_maps, core_ids=list(range(N_CORES)), trace=trace
    )
    out = np.concatenate([res.results[j]["ys"] for j in range(N_CORES)], axis=0)
    return out, res


def kernel(x, gamma, beta, weight, bias):
    out, _ = _run(x, gamma, beta, weight, bias, trace=False)
    return out
